# revision 12
# baseline (speedup 1.0000x reference)
"""DenseDilatedKnnGraph (B=2, C=128, N=8192, k=9, dilation=2) on 8 trn2 NeuronCores.

Strategy (row-block kNN, FAISS-style):
  - Host: L2-normalize x along C (fp64 -> fp32). All points are then unit
    norm, so ranking by squared euclidean distance == ranking by descending
    inner product; the device computes Q.T @ P (true-fp32 matmul) and, per
    1024-wide chunk of each row, the top-8 values + indices via DVE
    max/max_index. Values + chunk-local indices of the 64 candidates per row
    are shipped out.
  - Shard: 8 cores = 2 batches x 4 query-row blocks of 2048. Each core gets
    all 8192 points of its batch (columns) + its 2048 query rows.
  - Host merge (the FAISS shard-merge step): stable argsort of the 64
    candidates per row -> top-24 slots; slot -> chunk-local -> global index;
    dilation [::2]; center-index plane.
  - Exactness guard: a chunk can hide a true top-18 member only if all 8 of
    its candidates rank above the 18th-best candidate, i.e. the chunk owns 8
    of the top-24 slots. Detected host-side from the slots alone; flagged
    rows are recomputed exactly (fp64), so the result is exact for any input.
"""

import numpy as np

B, C, N = 2, 128, 8192
K = 9
K_CAND = 18
CHUNK = 1024
NCH = N // CHUNK          # 8 chunks
NCAND = NCH * 8           # 64 candidates per row
NQ_CORE = N // 4          # 2048 query rows per core
NT = NQ_CORE // 128       # 16 row-tiles per core
EPS = 1e-12

_CACHED_NC = None


def _build_nc():
    global _CACHED_NC
    if _CACHED_NC is not None:
        return _CACHED_NC
    import concourse.bacc as bacc
    import concourse.mybir as mybir
    from concourse.tile import TileContext

    nc = bacc.Bacc("TRN2", target_bir_lowering=False, debug=False)
    pq_in = nc.dram_tensor("pq", [128, N + NQ_CORE], mybir.dt.float32,
                           kind="ExternalInput")
    u_out = nc.dram_tensor("uo", [128, NT * NCAND], mybir.dt.uint16,
                           kind="ExternalOutput")
    f_out = nc.dram_tensor("fo", [128, NT * NCAND], mybir.dt.float32,
                           kind="ExternalOutput")

    with TileContext(nc) as tc:
        with (
            tc.tile_pool(name="const", bufs=1) as const_pool,
            tc.tile_pool(name="s", bufs=3) as s_pool,
            tc.tile_pool(name="psum", bufs=8, space="PSUM") as psum_pool,
        ):
            # layout: [Q (2048) | P (8192)]; staged DMAs so the first matmuls
            # start well before the full input lands. Each matmul waits on
            # exactly one DMA sem (fp32 self-loading matmul allows only one).
            S1 = NQ_CORE + CHUNK          # Q + P chunk 0
            S2 = NQ_CORE + N // 2         # + P chunks 1..3
            PQ = const_pool.tile([128, N + NQ_CORE], mybir.dt.float32)
            nc.sync.dma_start(PQ[:, :S1], pq_in[:, :S1])
            nc.sync.dma_start(PQ[:, S1:S2], pq_in[:, S1:S2])
            nc.sync.dma_start(PQ[:, S2:], pq_in[:, S2:])
            Q = PQ[:, :NQ_CORE]
            P = PQ[:, NQ_CORE:]

            UO = const_pool.tile([128, NT * NCAND], mybir.dt.uint16)
            FO = const_pool.tile([128, NT * NCAND], mybir.dt.float32)

            for t in range(NT):
                # one S tile per 1024-chunk -> finer dataflow (DVE starts on
                # chunk 0 as soon as its two 512-col blocks are copied).
                Sc = [s_pool.tile([128, CHUNK], mybir.dt.float32,
                                  name=f"s{t}_{ch}", tag=f"s{ch}")
                      for ch in range(NCH)]
                for blk in range(N // 512):
                    ps = psum_pool.tile([128, 512], mybir.dt.float32, tag="ps")
                    nc.tensor.matmul(ps[:], Q[:, t * 128:(t + 1) * 128],
                                     P[:, blk * 512:(blk + 1) * 512],
                                     start=True, stop=True)
                    S = Sc[blk // 2]
                    off = (blk % 2) * 512
                    nc.scalar.copy(S[:, off:off + 512], ps[:])

                CI = UO[:, t * NCAND: (t + 1) * NCAND]
                CV = FO[:, t * NCAND: (t + 1) * NCAND]
                for ch in range(NCH):
                    nc.vector.max(CV[:, ch * 8:(ch + 1) * 8], Sc[ch][:, :])
                    nc.vector.max_index(CI[:, ch * 8:(ch + 1) * 8],
                                        CV[:, ch * 8:(ch + 1) * 8],
                                        Sc[ch][:, :])

            nc.gpsimd.dma_start(u_out[:], UO[:])
            nc.gpsimd.dma_start(f_out[:], FO[:])

    nc.compile()
    _CACHED_NC = nc
    return nc


def _prep(x):
    x = np.asarray(x)
    xs = x[..., 0].astype(np.float64)                     # (B, C, N)
    norm = np.sqrt((xs * xs).sum(axis=1, keepdims=True))
    pts = (xs / np.maximum(norm, EPS)).astype(np.float32)  # (B, C, N) fp32
    in_maps = []
    for c in range(8):
        b, q = c // 4, c % 4
        qts = pts[b][:, q * NQ_CORE:(q + 1) * NQ_CORE]
        in_maps.append({"pq": np.ascontiguousarray(
            np.concatenate([qts, pts[b]], axis=1))})
    return pts, in_maps


def _exact_rows(pts_b, rows):
    """Exact fp64->fp32 top-K (dilated) for query rows of one batch,
    matching the reference ranking (ascending distance, ties by index)."""
    p64 = pts_b.astype(np.float64)                        # (C, N)
    s = (p64[:, rows].T @ p64).astype(np.float32)         # (R, N)
    order = np.argsort(-s, axis=1, kind="stable")
    return order[:, 0:K_CAND:2].astype(np.int32)


def _assemble(results, pts):
    nn = np.empty((B, N, K), np.int32)
    for c in range(8):
        b, q = c // 4, c % 4
        ci = results[c]["uo"].reshape(128, NT, NCAND).astype(np.int32)
        cv = results[c]["fo"].reshape(128, NT, NCAND)
        # shard-merge: top-24 candidate slots, value desc, slot asc on ties
        # (matches jax.lax.top_k tie order since slot order == index order)
        i2 = np.argsort(-cv, axis=2, kind="stable")[:, :, :24]
        s2 = i2[:, :, 0:K_CAND:2].astype(np.int32)  # ranks 0,2,...,16 -> 9
        loc = np.take_along_axis(ci, s2, axis=2)
        g = (s2 >> 3) * CHUNK + loc                 # global point index
        nn[b, q * NQ_CORE:(q + 1) * NQ_CORE] = \
            g.transpose(1, 0, 2).reshape(NQ_CORE, K)

        # conservative miss detection: some chunk owns 8 of the top-24 slots
        ch24 = i2 >> 3                              # [128, NT, 24] chunk ids
        susp = None
        for chn in range(NCH):
            cnt = (ch24 == chn).sum(axis=2) >= 8
            susp = cnt if susp is None else (susp | cnt)
        if susp.any():
            r_, t_ = np.nonzero(susp)
            rows = (q * NQ_CORE + t_ * 128 + r_).astype(np.int64)
            nn[b, rows] = _exact_rows(pts[b], rows)
    center = np.broadcast_to(
        np.arange(N, dtype=np.int32)[None, :, None], (B, N, K))
    return np.ascontiguousarray(
        np.stack([nn, center], axis=0).astype(np.int32))


def kernel(x):
    from concourse.bass_utils import run_bass_kernel_spmd
    nc = _build_nc()
    pts, in_maps = _prep(x)
    res = run_bass_kernel_spmd(nc, in_maps, core_ids=list(range(8)))
    return _assemble(res.results, pts)


def kernel_profiled(x):
    """Like kernel() but also returns the profiled HW execution time in ns."""
    from concourse.bass_utils import run_bass_kernel_spmd
    nc = _build_nc()
    pts, in_maps = _prep(x)
    res = run_bass_kernel_spmd(nc, in_maps, core_ids=list(range(8)), trace=True)
    return _assemble(res.results, pts), res.exec_time_ns


# revision 14
# speedup vs baseline: 1.0042x; 1.0042x over previous
"""DenseDilatedKnnGraph (B=2, C=128, N=8192, k=9, dilation=2) on 8 trn2 NeuronCores.

Strategy (row-block kNN, FAISS-style):
  - Host: L2-normalize x along C (fp64 -> fp32). All points are then unit
    norm, so ranking by squared euclidean distance == ranking by descending
    inner product; the device computes Q.T @ P (true-fp32 matmul) and, per
    1024-wide chunk of each row, the top-8 values + indices via DVE
    max/max_index. Values + chunk-local indices of the 64 candidates per row
    are shipped out.
  - Shard: 8 cores = 2 batches x 4 query-row blocks of 2048. Each core gets
    all 8192 points of its batch (columns) + its 2048 query rows.
  - Host merge (the FAISS shard-merge step): stable argsort of the 64
    candidates per row -> top-24 slots; slot -> chunk-local -> global index;
    dilation [::2]; center-index plane.
  - Exactness guard: a chunk can hide a true top-18 member only if all 8 of
    its candidates rank above the 18th-best candidate, i.e. the chunk owns 8
    of the top-24 slots. Detected host-side from the slots alone; flagged
    rows are recomputed exactly (fp64), so the result is exact for any input.
"""

import numpy as np

B, C, N = 2, 128, 8192
K = 9
K_CAND = 18
CHUNK = 1024
NCH = N // CHUNK          # 8 chunks
NCAND = NCH * 8           # 64 candidates per row
NQ_CORE = N // 4          # 2048 query rows per core
NT = NQ_CORE // 128       # 16 row-tiles per core
EPS = 1e-12

_CACHED_NC = None


def _build_nc():
    global _CACHED_NC
    if _CACHED_NC is not None:
        return _CACHED_NC
    import concourse.bacc as bacc
    import concourse.mybir as mybir
    from concourse.tile import TileContext

    nc = bacc.Bacc("TRN2", target_bir_lowering=False, debug=False)
    pq_in = nc.dram_tensor("pq", [128, N + NQ_CORE], mybir.dt.float32,
                           kind="ExternalInput")
    u_out = nc.dram_tensor("uo", [128, NT * NCAND], mybir.dt.uint16,
                           kind="ExternalOutput")
    f_out = nc.dram_tensor("fo", [128, NT * NCAND], mybir.dt.float32,
                           kind="ExternalOutput")

    with TileContext(nc) as tc:
        with (
            tc.tile_pool(name="const", bufs=1) as const_pool,
            tc.tile_pool(name="s", bufs=4) as s_pool,
            tc.tile_pool(name="psum", bufs=8, space="PSUM") as psum_pool,
        ):
            # layout: [Q (2048) | P (8192)]; staged DMAs so the first matmuls
            # start well before the full input lands. Each matmul waits on
            # exactly one DMA sem (fp32 self-loading matmul allows only one).
            S1 = NQ_CORE + CHUNK          # Q + P chunk 0
            S2 = NQ_CORE + N // 2         # + P chunks 1..3
            PQ = const_pool.tile([128, N + NQ_CORE], mybir.dt.float32)
            nc.gpsimd.dma_start(PQ[:, :S1], pq_in[:, :S1])
            nc.gpsimd.dma_start(PQ[:, S1:S2], pq_in[:, S1:S2])
            nc.gpsimd.dma_start(PQ[:, S2:], pq_in[:, S2:])
            Q = PQ[:, :NQ_CORE]
            P = PQ[:, NQ_CORE:]

            UO = const_pool.tile([128, NT * NCAND], mybir.dt.uint16)
            FO = const_pool.tile([128, NT * NCAND], mybir.dt.float32)

            for t in range(NT):
                # one S tile per 1024-chunk -> finer dataflow (DVE starts on
                # chunk 0 as soon as its two 512-col blocks are copied).
                Sc = [s_pool.tile([128, CHUNK], mybir.dt.float32,
                                  name=f"s{t}_{ch}", tag=f"s{ch}")
                      for ch in range(NCH)]
                for blk in range(N // 512):
                    ps = psum_pool.tile([128, 512], mybir.dt.float32, tag="ps")
                    nc.tensor.matmul(ps[:], Q[:, t * 128:(t + 1) * 128],
                                     P[:, blk * 512:(blk + 1) * 512],
                                     start=True, stop=True)
                    S = Sc[blk // 2]
                    off = (blk % 2) * 512
                    nc.scalar.copy(S[:, off:off + 512], ps[:])

                CI = UO[:, t * NCAND: (t + 1) * NCAND]
                CV = FO[:, t * NCAND: (t + 1) * NCAND]
                for ch in range(NCH):
                    nc.vector.max(CV[:, ch * 8:(ch + 1) * 8], Sc[ch][:, :])
                    nc.vector.max_index(CI[:, ch * 8:(ch + 1) * 8],
                                        CV[:, ch * 8:(ch + 1) * 8],
                                        Sc[ch][:, :])

            H = (NT // 2) * NCAND
            nc.gpsimd.dma_start(u_out[:, :H], UO[:, :H])
            nc.gpsimd.dma_start(f_out[:, :H], FO[:, :H])
            nc.gpsimd.dma_start(u_out[:, H:], UO[:, H:])
            nc.gpsimd.dma_start(f_out[:, H:], FO[:, H:])

    nc.compile()
    _CACHED_NC = nc
    return nc


def _prep(x):
    x = np.asarray(x)
    xs = x[..., 0].astype(np.float64)                     # (B, C, N)
    norm = np.sqrt((xs * xs).sum(axis=1, keepdims=True))
    pts = (xs / np.maximum(norm, EPS)).astype(np.float32)  # (B, C, N) fp32
    in_maps = []
    for c in range(8):
        b, q = c // 4, c % 4
        qts = pts[b][:, q * NQ_CORE:(q + 1) * NQ_CORE]
        in_maps.append({"pq": np.ascontiguousarray(
            np.concatenate([qts, pts[b]], axis=1))})
    return pts, in_maps


def _exact_rows(pts_b, rows):
    """Exact fp64->fp32 top-K (dilated) for query rows of one batch,
    matching the reference ranking (ascending distance, ties by index)."""
    p64 = pts_b.astype(np.float64)                        # (C, N)
    s = (p64[:, rows].T @ p64).astype(np.float32)         # (R, N)
    order = np.argsort(-s, axis=1, kind="stable")
    return order[:, 0:K_CAND:2].astype(np.int32)


def _assemble(results, pts):
    nn = np.empty((B, N, K), np.int32)
    for c in range(8):
        b, q = c // 4, c % 4
        ci = results[c]["uo"].reshape(128, NT, NCAND).astype(np.int32)
        cv = results[c]["fo"].reshape(128, NT, NCAND)
        # shard-merge: top-24 candidate slots, value desc, slot asc on ties
        # (matches jax.lax.top_k tie order since slot order == index order)
        i2 = np.argsort(-cv, axis=2, kind="stable")[:, :, :24]
        s2 = i2[:, :, 0:K_CAND:2].astype(np.int32)  # ranks 0,2,...,16 -> 9
        loc = np.take_along_axis(ci, s2, axis=2)
        g = (s2 >> 3) * CHUNK + loc                 # global point index
        nn[b, q * NQ_CORE:(q + 1) * NQ_CORE] = \
            g.transpose(1, 0, 2).reshape(NQ_CORE, K)

        # conservative miss detection: some chunk owns 8 of the top-24 slots
        ch24 = i2 >> 3                              # [128, NT, 24] chunk ids
        susp = None
        for chn in range(NCH):
            cnt = (ch24 == chn).sum(axis=2) >= 8
            susp = cnt if susp is None else (susp | cnt)
        if susp.any():
            r_, t_ = np.nonzero(susp)
            rows = (q * NQ_CORE + t_ * 128 + r_).astype(np.int64)
            nn[b, rows] = _exact_rows(pts[b], rows)
    center = np.broadcast_to(
        np.arange(N, dtype=np.int32)[None, :, None], (B, N, K))
    return np.ascontiguousarray(
        np.stack([nn, center], axis=0).astype(np.int32))


def kernel(x):
    from concourse.bass_utils import run_bass_kernel_spmd
    nc = _build_nc()
    pts, in_maps = _prep(x)
    res = run_bass_kernel_spmd(nc, in_maps, core_ids=list(range(8)))
    return _assemble(res.results, pts)


def kernel_profiled(x):
    """Like kernel() but also returns the profiled HW execution time in ns."""
    from concourse.bass_utils import run_bass_kernel_spmd
    nc = _build_nc()
    pts, in_maps = _prep(x)
    res = run_bass_kernel_spmd(nc, in_maps, core_ids=list(range(8)), trace=True)
    return _assemble(res.results, pts), res.exec_time_ns
